# revision 6
# baseline (speedup 1.0000x reference)
"""BlockDWT2d (3-level Haar wavelet packet, 8x8 blocks) on 8 TRN2 NeuronCores.

Input  x: (32, 3, 512, 512) fp32 -> output (32, 192, 64, 64) fp32.

Math: the 3-level full packet transform is separable: for each 8x8 input
block, out2d = (H8/8) @ X8 @ H8^T where H8 is the natural-binary-order
Hadamard matrix; output channel k_sub bit-interleaves the row/col
transform indices (kH, kW): k_sub = 32h2+16w2+8h1+4w1+2h0+w0, and the
full channel index is K = 3*k_sub + c.

Per-core pipeline (batch-sharded 4 samples/core, 12 images of 512x512):
  DMA in:  X [p=h(128) x4 tiles, f=w(512)] (contiguous, full-rate)
  Pass A   (per t,q): psum = X[:, t, 128q:128q+128]^T @ BD1
           BD1[(g,i),(kH,g')] = H8[kH,i]/8 * delta_gg'
           -> A[p=(xbl,j), f=(kH,g)];  copy/regroup -> Bq[p,(kH,yb)]
  Pass B   (per q,r): psum = Bq[:, 128r:128r+128]^T @ BD2
           BD2[(xbl,j),(kW,xbl')] = H8[kW,j] * delta_xbl
           -> C[p=(h0,yb), f=(kW,xbl)]; copy/regroup -> Fr[p,(kW,xb)]
  DMA out: Fr -> out[b, K(kH,kW)*3+c, yb, xb] (256B runs)
"""

import numpy as np

_CACHE = {}


def _h8():
    h = np.ones((8, 8), np.float32)
    x = np.eye(8, dtype=np.float32).reshape(1, 8, 8)
    for _ in range(3):
        a, b = x[:, 0::2, :], x[:, 1::2, :]
        x = np.concatenate([a + b, a - b], axis=0)
    return x[:, 0, :]  # H8[k, i], entries +-1


def _constants():
    H8 = _h8()
    bd1 = np.zeros((128, 128), np.float32)  # [(g,i), (kH,g')]
    for g in range(16):
        bd1[g * 8:(g + 1) * 8, :].reshape(8, 8, 16)[:, :, g] = (H8.T / 8.0)
    bd2 = np.zeros((128, 128), np.float32)  # [(xbl,j), (kW,xbl')]
    for xbl in range(16):
        bd2[xbl * 8:(xbl + 1) * 8, :].reshape(8, 8, 16)[:, :, xbl] = H8.T
    return bd1, bd2


def _build_nc():
    from contextlib import ExitStack

    import concourse.bass as bass
    import concourse.tile as tile
    from concourse import bacc, mybir

    nc = bacc.Bacc("TRN2", target_bir_lowering=False, debug=False)

    x_in = nc.dram_tensor("x", [12, 512, 512], mybir.dt.float32,
                          kind="ExternalInput")
    bd1_d = nc.dram_tensor("bd1", [128, 128], mybir.dt.float32,
                           kind="ExternalInput")
    bd2_d = nc.dram_tensor("bd2", [128, 128], mybir.dt.float32,
                           kind="ExternalInput")
    out_d = nc.dram_tensor("out", [4, 192, 64, 64], mybir.dt.float32,
                           kind="ExternalOutput")

    # out view: [b, h2, w2, h1, w1, h0, w0, c, yb, xb]
    ov = out_d.ap().rearrange(
        "bb (h2 w2 h1 w1 h0 w0 c) yb xb -> bb h2 w2 h1 w1 h0 w0 c yb xb",
        h2=2, w2=2, h1=2, w1=2, h0=2, w0=2, c=3)

    with tile.TileContext(nc) as tc, ExitStack() as ctx:
        cpool = ctx.enter_context(tc.tile_pool(name="consts", bufs=1))
        xpool = ctx.enter_context(tc.tile_pool(name="xin", bufs=2))
        bpool = ctx.enter_context(tc.tile_pool(name="bq", bufs=9))
        fpool = ctx.enter_context(tc.tile_pool(name="fr", bufs=9))
        ppool = ctx.enter_context(tc.tile_pool(name="ps", bufs=6, space="PSUM"))

        bd1_s = cpool.tile([128, 128], mybir.dt.float32, tag="bd1")
        bd2_s = cpool.tile([128, 128], mybir.dt.float32, tag="bd2")
        nc.sync.dma_start(bd1_s[:], bd1_d.ap())
        nc.sync.dma_start(bd2_s[:], bd2_d.ap())

        ncopy = 0
        for img in range(12):
            b, c = img // 3, img % 3
            xt = xpool.tile([128, 4, 512], mybir.dt.float32, tag="x")
            nc.sync.dma_start(
                xt[:], x_in.ap()[img].rearrange("(t p) w -> p t w", p=128))

            bqs = []
            for q in range(4):
                bq = bpool.tile([128, 512], mybir.dt.float32, tag="bq",
                                name=f"bq_{img}_{q}")
                bqs.append(bq)
            for t in range(4):
                for q in range(4):
                    ps = ppool.tile([128, 128], mybir.dt.float32, tag="ps",
                                    name=f"psA_{img}_{t}_{q}")
                    nc.tensor.matmul(ps[:],
                                     lhsT=xt[:, t, q * 128:(q + 1) * 128],
                                     rhs=bd1_s[:], start=True, stop=True)
                    dst = bqs[q].rearrange("p (a t g) -> p a t g",
                                           a=8, t=4)[:, :, t, :]
                    src = ps.rearrange("p (a g) -> p a g", a=8)
                    if ncopy % 2 == 0:
                        nc.vector.tensor_copy(dst, src)
                    else:
                        nc.scalar.copy(dst, src)
                    ncopy += 1

            frs = []
            for r in range(4):
                fr = fpool.tile([128, 512], mybir.dt.float32, tag="fr",
                                name=f"fr_{img}_{r}")
                frs.append(fr)
            for q in range(4):
                for r in range(4):
                    ps = ppool.tile([128, 128], mybir.dt.float32, tag="ps",
                                    name=f"psB_{img}_{q}_{r}")
                    nc.tensor.matmul(ps[:],
                                     lhsT=bqs[q][:, r * 128:(r + 1) * 128],
                                     rhs=bd2_s[:], start=True, stop=True)
                    dst = frs[r].rearrange("p (a q g) -> p a q g",
                                           a=8, q=4)[:, :, q, :]
                    src = ps.rearrange("p (a g) -> p a g", a=8)
                    if ncopy % 2 == 0:
                        nc.vector.tensor_copy(dst, src)
                    else:
                        nc.scalar.copy(dst, src)
                    ncopy += 1

            for r in range(4):
                h2, h1 = r // 2, r % 2
                for kw in range(8):
                    w2, w1, w0 = kw // 4, (kw // 2) % 2, kw % 2
                    # dst dims after slice: (h0, yb, xb) — matches src enum
                    dst = ov[b, h2, w2, h1, w1, :, w0, c, :, :]
                    nc.sync.dma_start(dst,
                                      frs[r][:, kw * 64:(kw + 1) * 64])

    nc.compile()
    return nc


def _get_nc():
    if "nc" not in _CACHE:
        _CACHE["nc"] = _build_nc()
    return _CACHE["nc"]


def kernel(x: np.ndarray) -> np.ndarray:
    from concourse.bass_utils import run_bass_kernel_spmd

    assert x.shape == (32, 3, 512, 512) and x.dtype == np.float32
    nc = _get_nc()
    bd1, bd2 = _constants()
    in_maps = []
    for i in range(8):
        shard = np.ascontiguousarray(
            x[4 * i:4 * i + 4].reshape(12, 512, 512))
        in_maps.append({"x": shard, "bd1": bd1, "bd2": bd2})
    res = run_bass_kernel_spmd(nc, in_maps, core_ids=list(range(8)))
    return np.concatenate([r["out"] for r in res.results], axis=0)


# revision 18
# speedup vs baseline: 1.0558x; 1.0558x over previous
"""BlockDWT2d (3-level Haar wavelet packet, 8x8 blocks) on 8 TRN2 NeuronCores.

Input  x: (32, 3, 512, 512) fp32 -> output (32, 192, 64, 64) fp32.

Math: the 3-level full packet transform is separable: for each 8x8 input
block, out2d = (H8/8) @ X8 @ H8^T where H8 is the natural-binary-order
Hadamard matrix; output channel k_sub bit-interleaves the row/col
transform indices (kH, kW): k_sub = 32h2+16w2+8h1+4w1+2h0+w0, and the
full channel index is K = 3*k_sub + c.

Per-core pipeline (batch-sharded 4 samples/core, 12 images of 512x512):
  DMA in:  X [p=h(128) x4 tiles, f=w(512)] (contiguous, full-rate HWDGE)
  Pass A   (per q: 4 mms t=0..3): psA[:, 128t:] = X[:,t,128q:]^T @ BD1
           BD1[(g,i),(kH,g')] = H8[kH,i]/8 * delta_gg'
           -> [p=(xbl,j), f=(t,kH,g)]; one copy/regroup -> Bq[p,(kH,yb)]
  Pass B   (per r: 4 mms q=0..3): psB[:, 128q:] = Bq[:, 128r:]^T @ BD2
           BD2[(xbl,j),(kW,xbl')] = H8[kW,j] * delta_xbl
           -> [p=(h0,yb), f=(q,kW,xbl)]; one copy/regroup -> Fr[p,(kW,xb)]
  DMA out: direct HWDGE DMAs, one per (img, r, kW): src [128p, 64f],
           DRAM dims (h0, yb, xb) — 3-dim AP, 256B runs. Alternated
           between the SP and ACT HWDGE rings. Input loads go through
           GPSIMD SWDGE so they never queue behind output-DMA waits.
"""

import numpy as np

_CACHE = {}


def _h8():
    x = np.eye(8, dtype=np.float32).reshape(1, 8, 8)
    for _ in range(3):
        a, b = x[:, 0::2, :], x[:, 1::2, :]
        x = np.concatenate([a + b, a - b], axis=0)
    return x[:, 0, :]  # H8[k, i], entries +-1


def _interleave(kH, kW):
    h2, h1, h0 = (kH >> 2) & 1, (kH >> 1) & 1, kH & 1
    w2, w1, w0 = (kW >> 2) & 1, (kW >> 1) & 1, kW & 1
    return 32 * h2 + 16 * w2 + 8 * h1 + 4 * w1 + 2 * h0 + 1 * w0


def _constants():
    H8 = _h8()
    bd1 = np.zeros((128, 128), np.float32)  # [(g,i), (kH,g')]
    for g in range(16):
        bd1[g * 8:(g + 1) * 8, :].reshape(8, 8, 16)[:, :, g] = (H8.T / 8.0)
    bd2 = np.zeros((128, 128), np.float32)  # [(xbl,j), (kW,xbl')]
    for xbl in range(16):
        bd2[xbl * 8:(xbl + 1) * 8, :].reshape(8, 8, 16)[:, :, xbl] = H8.T
    return {"bd1": bd1, "bd2": bd2}


def _build_nc():
    from contextlib import ExitStack

    import concourse.bass as bass
    import concourse.tile as tile
    from concourse import bacc, mybir

    nc = bacc.Bacc("TRN2", target_bir_lowering=False, debug=False)

    x_in = nc.dram_tensor("x", [12, 512, 512], mybir.dt.float32,
                          kind="ExternalInput")
    bd1_d = nc.dram_tensor("bd1", [128, 128], mybir.dt.float32,
                           kind="ExternalInput")
    bd2_d = nc.dram_tensor("bd2", [128, 128], mybir.dt.float32,
                           kind="ExternalInput")
    out_d = nc.dram_tensor("out", [4, 192, 64, 64], mybir.dt.float32,
                           kind="ExternalOutput")
    # out view: [b, h2, w2, h1, w1, h0, w0, c, yb, xb]
    ov = out_d.ap().rearrange(
        "bb (h2 w2 h1 w1 h0 w0 c) yb xb -> bb h2 w2 h1 w1 h0 w0 c yb xb",
        h2=2, w2=2, h1=2, w1=2, h0=2, w0=2, c=3)

    with tile.TileContext(nc) as tc, ExitStack() as ctx:
        cpool = ctx.enter_context(tc.tile_pool(name="consts", bufs=1))
        xpool = ctx.enter_context(tc.tile_pool(name="xin", bufs=2))
        bpool = ctx.enter_context(tc.tile_pool(name="bq", bufs=9))
        fpool = ctx.enter_context(tc.tile_pool(name="fr", bufs=9))
        ppool = ctx.enter_context(tc.tile_pool(name="ps", bufs=6, space="PSUM"))

        bd1_s = cpool.tile([128, 128], mybir.dt.float32, tag="bd1")
        bd2_s = cpool.tile([128, 128], mybir.dt.float32, tag="bd2")
        nc.gpsimd.dma_start(bd1_s[:], bd1_d.ap())
        nc.gpsimd.dma_start(bd2_s[:], bd2_d.ap())

        ndma = 0
        for img in range(12):
            xt = xpool.tile([128, 4, 512], mybir.dt.float32, tag="x")
            nc.gpsimd.dma_start(
                xt[:], x_in.ap()[img].rearrange("(t p) w -> p t w", p=128))

            bqs = []
            for q in range(4):
                psa = ppool.tile([128, 512], mybir.dt.float32, tag="ps",
                                 name=f"psA_{img}_{q}")
                for t in range(4):
                    nc.tensor.matmul(psa[:, t * 128:(t + 1) * 128],
                                     lhsT=xt[:, t, q * 128:(q + 1) * 128],
                                     rhs=bd1_s[:], start=True, stop=True)
                bq = bpool.tile([128, 512], mybir.dt.float32, tag="bq",
                                name=f"bq_{img}_{q}")
                dst = bq.rearrange("p (a t g) -> p t a g", a=8, t=4)
                src = psa.rearrange("p (t a g) -> p t a g", t=4, a=8)
                nc.vector.tensor_copy(dst, src)
                bqs.append(bq)

            b, c = img // 3, img % 3
            for r in range(4):
                h2, h1 = r // 2, r % 2
                psb = ppool.tile([128, 512], mybir.dt.float32, tag="ps",
                                 name=f"psB_{img}_{r}")
                for q in range(4):
                    nc.tensor.matmul(psb[:, q * 128:(q + 1) * 128],
                                     lhsT=bqs[q][:, r * 128:(r + 1) * 128],
                                     rhs=bd2_s[:], start=True, stop=True)
                fr = fpool.tile([128, 512], mybir.dt.float32, tag="fr",
                                name=f"fr_{img}_{r}")
                dst = fr.rearrange("p (a q g) -> p q a g", a=8, q=4)
                src = psb.rearrange("p (q a g) -> p q a g", q=4, a=8)
                nc.vector.tensor_copy(dst, src)

                for kw in range(8):
                    w2, w1, w0 = kw // 4, (kw // 2) % 2, kw % 2
                    # dst dims (h0, yb, xb) matches src enumeration
                    dma_dst = ov[b, h2, w2, h1, w1, :, w0, c, :, :]
                    eng = nc.sync if ndma % 2 == 0 else nc.scalar
                    eng.dma_start(dma_dst, fr[:, kw * 64:(kw + 1) * 64])
                    ndma += 1

    nc.compile()
    return nc


def _get_nc():
    if "nc" not in _CACHE:
        _CACHE["nc"] = _build_nc()
    return _CACHE["nc"]


def kernel(x: np.ndarray) -> np.ndarray:
    from concourse.bass_utils import run_bass_kernel_spmd

    assert x.shape == (32, 3, 512, 512) and x.dtype == np.float32
    nc = _get_nc()
    consts = _constants()
    in_maps = []
    for i in range(8):
        shard = np.ascontiguousarray(
            x[4 * i:4 * i + 4].reshape(12, 512, 512))
        in_maps.append({"x": shard, **consts})
    res = run_bass_kernel_spmd(nc, in_maps, core_ids=list(range(8)))
    return np.concatenate([r["out"] for r in res.results], axis=0)
